# revision 34
# baseline (speedup 1.0000x reference)
"""AttentiveFPConv GNN message-passing kernel for 8 Trainium2 NeuronCores.

Reference computation (all fp32):
    alpha = sigmoid(x[col] @ Wa_w + Wa_b)          # per-edge attention
    neigh = x[col] * alpha                          # per-edge message
    aggr  = segment_sum(neigh, row, N)              # per-node aggregation
    out   = tanh(x @ Wn_w + Wn_b + aggr @ Wg_w + Wg_b)

Key algebraic identity: alpha depends only on the source node, so
    h = x * sigmoid(x @ Wa_w + Wa_b)                # per-NODE tensor
    aggr[n] = sum_{e: row[e]=n} h[col[e]]           # gather + segment-sum

Sharding: destination-node sharding. Core k owns nodes [5000k, 5000(k+1))
and ALL edges targeting them (balanced: rows are uniform). No collective
needed: each core computes its own aggr and output slice.

Per-core pipeline:
  Phase 1: h = x*sigmoid(x@Wa+b) for ALL nodes (replicated). h stored in
           HBM partition-major as [128, T, D] so the store DMA has large
           contiguous per-partition runs (full DMA rate); the gather
           addresses node c at row (c%128)*T + c//128.
  Phase 2: dma_gather h[col] in destination-sorted edge order (4 SWDGE
           queues, 512-idx chunks: the 1024-desc ring then pipelines two
           chunks per queue); segment-sum via one-hot matmuls accumulating
           aggr^T in PSUM per 128-node block. One-hot M is built ON-CHIP:
           DVE tensor_tensor is_equal of a bf16 iota against the per-edge
           local-row stream (bcast AP), 4 tiles per op.
           Edges are split into THREE streams by col range (h segments
           written in order), so gathers for stream s can start as soon
           as segment s is in HBM -- keeps the Pool engine (the SWDGE
           descriptor generator, the critical resource) continuously fed.
           Streams also keep every int16 gather index < 32768.
  Phase 3: out = tanh(x@Wn + aggr@Wg + ones x bias) -- bias added by a
           rank-1 matmul into the same PSUM accumulation group.
"""

import numpy as np
import ml_dtypes

BF16 = ml_dtypes.bfloat16

# ---------------------------------------------------------------- parameters

class P:
    """Problem/kernel parameters (full-size defaults; shrinkable for tests)."""
    def __init__(self, N=40000, D=128, NCORES=8, SPLITS=(4992, 22464),
                 GCHUNK=512, PH1_CHUNK=2048, NQ=4, SCRATCH=16384,
                 PIECE=4992, GBUFS=16, NEGPAD=False):
        assert D == 128
        self.N, self.D, self.NCORES = N, D, NCORES
        self.NB = N // NCORES                 # nodes per core
        self.GCHUNK = GCHUNK                  # idxs per dma_gather call
        self.GT = GCHUNK // 128               # gather tiles per chunk
        self.PH1_CHUNK = PH1_CHUNK            # nodes per phase-1 xT chunk
        self.NBLK = (self.NB + 127) // 128    # 128-node blocks per core
        self.NQ = NQ                          # SWDGE queues for dma_gather
        self.SCRATCH = SCRATCH                # SWDGE desc ring bytes/partition
        self.PIECE = PIECE                    # nodes per phase-1 staging piece
        self.GBUFS = GBUFS                    # gather tile-pool depth per stream
        self.NEGPAD = NEGPAD                  # use idx=-1 (skipped) for padding
        # h segments: [lo, hi) node ranges, each with partition-major tiles
        bounds = (0,) + tuple(SPLITS) + (N,)
        self.SEG = []
        for lo, hi in zip(bounds[:-1], bounds[1:]):
            T = (hi - lo + 127) // 128
            assert 127 * T + (T - 1) < 32768, "int16 gather index overflow"
            self.SEG.append((lo, hi, T))
        self.NSEG = len(self.SEG)


# ------------------------------------------------------------ host edge prep

def prep_edges(p: P, row: np.ndarray, col: np.ndarray):
    """Per-core destination-sorted, block-padded edge streams (one per h
    segment). Returns per-stream tile counts and per-core idx/lr arrays."""
    row = np.asarray(row).astype(np.int64)
    col = np.asarray(col).astype(np.int64)
    NS = p.NSEG
    cores = []
    for k in range(p.NCORES):
        sel = (row // p.NB) == k
        r = (row[sel] - k * p.NB).astype(np.int32)
        c = col[sel].astype(np.int32)
        order = np.argsort(r, kind="stable")
        r, c = r[order], c[order]
        lo = np.searchsorted(r, np.arange(p.NBLK) * 128)
        hi = np.searchsorted(r, np.minimum(np.arange(1, p.NBLK + 1) * 128, p.NB))
        blocks = []
        for b in range(p.NBLK):
            rb = r[lo[b]:hi[b]] - b * 128
            cb = c[lo[b]:hi[b]]
            per_stream = []
            for (slo, shi, T) in p.SEG:
                m = (cb >= slo) & (cb < shi)
                cs = cb[m] - slo
                # partition-major h layout: node c -> row (c%128)*T + c//128
                per_stream.append(((cs % 128) * T + cs // 128, rb[m]))
            blocks.append(per_stream)
        cores.append(blocks)

    cnt = np.array([[[len(cores[k][b][s][0]) for b in range(p.NBLK)]
                     for s in range(NS)] for k in range(p.NCORES)])
    tS = np.maximum(1, -(-cnt.max(axis=0) // 128))          # [NS, NBLK]
    L = [int(tS[s].sum()) * 128 for s in range(NS)]
    Lg = [-(-L[s] // p.GCHUNK) * p.GCHUNK for s in range(NS)]

    per_core = []
    for k in range(p.NCORES):
        m = {}
        for s in range(NS):
            # Padding slots use idx=-1 (descriptor skipped by the gather HW;
            # the matching one-hot row is all-zero so stale SBUF data is
            # harmless). The first GBUFS chunks instead pad with idx=0 so
            # every gather-pool buffer gets fully written once -- later
            # skipped slots then hold stale-but-finite h values, never
            # uninitialized SBUF (which could be NaN and poison 0*NaN).
            idx = np.full(Lg[s], -1, np.int16)
            first = min(Lg[s], p.GBUFS * p.GCHUNK) if p.NEGPAD else Lg[s]
            idx[:first] = 0
            lr = np.full(L[s], -1.0, np.float32)
            o = 0
            for b in range(p.NBLK):
                cs, rs = cores[k][b][s]
                idx[o:o + len(cs)] = cs
                lr[o:o + len(rs)] = rs
                o += int(tS[s][b]) * 128
            # A chunk with zero descriptors may never fire its completion
            # semaphore -- make sure every chunk gathers at least one row.
            ch = idx.reshape(-1, p.GCHUNK)
            empty = (ch >= 0).sum(axis=1) == 0
            ch[empty] = 0
            m[f"idx{s}"] = np.tile(idx.reshape(-1, 16).T, (8, 1))
            m[f"lr{s}"] = lr.reshape(-1, 128).T.astype(BF16)
        per_core.append(m)
    return tS, L, Lg, per_core


# ------------------------------------------------------------- device kernel

def build(p: P, tS, L, Lg):
    from concourse import bacc, mybir, tile

    f32, bf16, i16 = mybir.dt.float32, mybir.dt.bfloat16, mybir.dt.int16
    AF = mybir.ActivationFunctionType
    nc = bacc.Bacc("TRN2", target_bir_lowering=False, debug=False,
                   num_devices=p.NCORES, num_swdge_queues=p.NQ,
                   dynamic_dma_scratch_size=p.SCRATCH)

    N, D, NB, NBLK, NS = p.N, p.D, p.NB, p.NBLK, p.NSEG
    PIECE = p.PIECE
    S = [L[s] // 128 for s in range(NS)]      # one-hot tiles per stream

    xT_d   = nc.dram_tensor("xT", [D, N], bf16, kind="ExternalInput")
    xTo_d  = nc.dram_tensor("xT_own", [D, NB], bf16, kind="ExternalInput")
    WaW_d  = nc.dram_tensor("WaW", [D, D], bf16, kind="ExternalInput")
    WaB_d  = nc.dram_tensor("WaB", [D, 1], f32, kind="ExternalInput")
    WnW_d  = nc.dram_tensor("WnW", [D, D], bf16, kind="ExternalInput")
    WgW_d  = nc.dram_tensor("WgW", [D, D], bf16, kind="ExternalInput")
    bias_d = nc.dram_tensor("biasR", [1, D], bf16, kind="ExternalInput")
    ones_d = nc.dram_tensor("onesR", [1, D], bf16, kind="ExternalInput")
    ident_d= nc.dram_tensor("ident", [D, D], bf16, kind="ExternalInput")
    iota_d = nc.dram_tensor("iota4", [128, 512], bf16, kind="ExternalInput")
    iotaP_d = nc.dram_tensor("iotaP", [128, 128], bf16, kind="ExternalInput")
    idx_d  = [nc.dram_tensor(f"idx{s}", [128, Lg[s] // 16], i16,
                             kind="ExternalInput") for s in range(NS)]
    lr_d   = [nc.dram_tensor(f"lr{s}", [128, S[s]], bf16,
                             kind="ExternalInput") for s in range(NS)]
    out_d  = nc.dram_tensor("out", [NB, D], f32, kind="ExternalOutput")
    h_d    = [nc.dram_tensor(f"h{s}", [128, p.SEG[s][2], D], bf16,
                             kind="Internal") for s in range(NS)]
    h_rows = [h_d[s][:, :, :].rearrange("p t d -> (p t) d") for s in range(NS)]

    from contextlib import ExitStack
    with tile.TileContext(nc) as tc:
        with ExitStack() as stack:
            pool = lambda name, bufs, **kw: stack.enter_context(
                tc.tile_pool(name=name, bufs=bufs, **kw))
            cpool = pool("const", 1)
            xpool = pool("xchunk", 3)
            htpool = pool("hT", 2)
            hspool = pool("hstage", 2)
            pg_pool = pool("pg", 2, space="PSUM")
            pt_pool = pool("pt", 1, space="PSUM")
            pi_pool = pool("pi", 1, space="PSUM")
            pa_pool = pool("pa", 2, space="PSUM")
            po_pool = pool("po", 2, space="PSUM")
            g0pool = pool("s0", 16)
            g1pool = pool("s1", 16)
            g2pool = pool("s2", 16)
            mpool = pool("m", 8)
            agg0pool = pool("agg0", (NBLK + 3) // 4)
            agg1pool = pool("agg1", (NBLK + 3) // 4)
            agg2pool = pool("agg2", 3)
            w1pool = pool("ph1w", 4)
            ospool = pool("ostage", 2)
            gpools = [g0pool, g1pool, g2pool]
            aggpools = [agg0pool, agg1pool, agg2pool]

            # ---- constants needed by phase 1 (issued first on the SP queue)
            WaW = cpool.tile([D, D], bf16); nc.sync.dma_start(out=WaW[:], in_=WaW_d[:])
            WaB = cpool.tile([D, 1], f32); nc.sync.dma_start(out=WaB[:], in_=WaB_d[:])
            ident = cpool.tile([D, D], bf16); nc.sync.dma_start(out=ident[:], in_=ident_d[:])

            # phase-2/3 constants: issued on the Act HWDGE queue so they do
            # not delay the phase-1 x-chunk stream on the SP queue.
            WnW = cpool.tile([D, D], bf16); nc.scalar.dma_start(out=WnW[:], in_=WnW_d[:])
            WgW = cpool.tile([D, D], bf16); nc.scalar.dma_start(out=WgW[:], in_=WgW_d[:])
            biasR = cpool.tile([1, D], bf16); nc.scalar.dma_start(out=biasR[:], in_=bias_d[:])
            onesR = cpool.tile([1, D], bf16); nc.scalar.dma_start(out=onesR[:], in_=ones_d[:])
            iota4 = cpool.tile([128, 512], bf16); nc.scalar.dma_start(out=iota4[:], in_=iota_d[:])
            iotaP = cpool.tile([128, 128], bf16); nc.scalar.dma_start(out=iotaP[:], in_=iotaP_d[:])
            xT_own = cpool.tile([D, NB], bf16); nc.scalar.dma_start(out=xT_own[:], in_=xTo_d[:])
            lr_sb = []
            idx_sb = []
            for s in range(NS):
                lt = cpool.tile([128, S[s]], bf16, tag=f"lr{s}_sb")
                nc.scalar.dma_start(out=lt[:], in_=lr_d[s][:])
                lr_sb.append(lt)
                it = cpool.tile([128, Lg[s] // 16], i16, tag=f"idx{s}_sb")
                nc.scalar.dma_start(out=it[:], in_=idx_d[s][:])
                idx_sb.append(it)

            # ---- phase 1: h = x * sigmoid(x@Wa + b); hT pieces -> h[128,T,D]
            def ph1_compute(hTp, base, cn):
                """Compute hT for nodes [base, base+cn) into hTp[:, :cn]."""
                off = 0
                while off < cn:
                    w = min(p.PH1_CHUNK, cn - off)
                    xc = xpool.tile([D, p.PH1_CHUNK], bf16, tag="xc")
                    nc.sync.dma_start(out=xc[:, :w], in_=xT_d[:, base + off:base + off + w])
                    g0 = 0
                    while g0 < w:
                        gw = min(512, w - g0)
                        pg = pg_pool.tile([D, 512], f32, tag="pg")
                        nc.tensor.matmul(pg[:, :gw], lhsT=WaW[:],
                                         rhs=xc[:, g0:g0 + gw], start=True, stop=True)
                        sT = w1pool.tile([D, 512], bf16, tag="sT")
                        nc.scalar.activation(sT[:, :gw], pg[:, :gw], AF.Sigmoid,
                                             bias=WaB[:, 0:1])
                        nc.vector.tensor_tensor(out=hTp[:, off + g0:off + g0 + gw],
                                                in0=xc[:, g0:g0 + gw],
                                                in1=sT[:, :gw], op=mybir.AluOpType.mult)
                        g0 += gw
                    off += w

            def ph1_flush(hTp, h_t, t_base, cn):
                """PE-transpose hTp[:, :cn] into node-major tiles and DMA to
                h_t[:, t_base:t_base+ceil(cn/128), :] (partition-major)."""
                nfull = cn // 128
                rem = cn - nfull * 128
                if nfull:
                    hst = hspool.tile([128, PIECE // 128, 128], bf16, tag="hst")
                    t0 = 0
                    while t0 < nfull:
                        tn = min(4, nfull - t0)
                        pt = pt_pool.tile([128, 512], bf16, tag="pt")
                        for q in range(tn):
                            nc.tensor.transpose(
                                pt[:, q * 128:(q + 1) * 128],
                                hTp[:, (t0 + q) * 128:(t0 + q + 1) * 128], ident[:])
                        nc.vector.tensor_copy(
                            out=hst[:, t0:t0 + tn, :].rearrange("p t d -> p (t d)"),
                            in_=pt[:, :tn * 128])
                        t0 += tn
                    nc.sync.dma_start(out=h_t[:, t_base:t_base + nfull, :],
                                      in_=hst[:, :nfull, :])
                if rem:
                    pt = pt_pool.tile([128, 512], bf16, tag="pt")
                    nc.tensor.transpose(pt[:rem, :128], hTp[:, nfull * 128:nfull * 128 + rem],
                                        ident[:])
                    tl = w1pool.tile([128, 128], bf16, tag="tail")
                    nc.vector.tensor_copy(out=tl[:rem, :], in_=pt[:rem, :128])
                    nc.sync.dma_start(
                        out=h_t[:rem, t_base + nfull:t_base + nfull + 1, :],
                        in_=tl[:rem, :].rearrange("p (t d) -> p t d", t=1))

            for s in range(NS):
                slo, shi, T = p.SEG[s]
                base = slo
                while base < shi:
                    cn = min(PIECE, shi - base)
                    hTp = htpool.tile([D, PIECE], bf16, tag="hT")
                    ph1_compute(hTp, base, cn)
                    ph1_flush(hTp, h_d[s], (base - slo) // 128, cn)
                    base += cn

            # ---- phase 2: NS scatter passes (stream s gathers from h_d[s])
            nq_counter = [0]
            g_tiles = [[None] * (Lg[s] // p.GCHUNK) for s in range(NS)]
            MGT = 4
            m_tiles = [[None] * (-(-S[s] // MGT)) for s in range(NS)]

            def ensure_chunk(s, ci):
                if g_tiles[s][ci] is not None:
                    return
                g = gpools[s].tile([128, p.GT, D], bf16, tag=f"g{s}")
                c0 = ci * (p.GCHUNK // 16)
                nc.gpsimd.dma_gather(
                    out_ap=g[:], in_ap=h_rows[s],
                    idxs_ap=idx_sb[s][:, c0:c0 + p.GCHUNK // 16],
                    num_idxs=p.GCHUNK, num_idxs_reg=p.GCHUNK, elem_size=D,
                    queue_num=nq_counter[0] % p.NQ)
                nq_counter[0] += 1
                g_tiles[s][ci] = g

            def ensure_mchunk(s, ci):
                """Build 4 one-hot tiles on-chip: M[e,d] = (lr[e] == d)."""
                if m_tiles[s][ci] is not None:
                    return
                t0 = ci * MGT
                tn = min(MGT, S[s] - t0)
                mt = mpool.tile([128, MGT, D], bf16, tag=f"m{s}")
                nc.vector.tensor_tensor(
                    out=mt[:, :tn, :],
                    in0=iota4[:, :tn * 128].rearrange("p (t d) -> p t d", d=128),
                    in1=lr_sb[s][:, t0:t0 + tn].broadcast_to([128, tn, 128]),
                    op=mybir.AluOpType.is_equal)
                m_tiles[s][ci] = mt

            pos = [np.concatenate([[0], np.cumsum(tS[s])]).astype(int)
                   for s in range(NS)]

            def scatter_group(s, b0, gn, pa):
                """Accumulate stream s's one-hot matmuls for blocks
                [b0, b0+gn) into PSUM pa."""
                for q in range(gn):
                    b = b0 + q
                    tcnt = int(tS[s][b])
                    for j in range(tcnt):
                        g = pos[s][b] + j
                        ensure_chunk(s, g // p.GT)
                        ensure_mchunk(s, g // MGT)
                        neigh = g_tiles[s][g // p.GT][:, g % p.GT, :]
                        M = m_tiles[s][g // MGT][:, g % MGT, :]
                        nc.tensor.matmul(pa[:, q * 128:(q + 1) * 128],
                                         lhsT=neigh, rhs=M,
                                         start=(j == 0), stop=(j == tcnt - 1))

            # passes 0..NS-2 buffered in SBUF
            aggs = []
            for s in range(NS - 1):
                cur = []
                b0 = 0
                while b0 < NBLK:
                    gn = min(4, NBLK - b0)
                    pa = pa_pool.tile([D, 512], f32, tag="pa")
                    scatter_group(s, b0, gn, pa)
                    agg = aggpools[s].tile([D, 512], bf16, tag=f"agg{s}")
                    nc.vector.tensor_copy(out=agg[:], in_=pa[:])
                    cur.append(agg)
                    b0 += gn
                aggs.append(cur)

            # ---- last pass + phase 3 fused per 4-block group
            sl_ = NS - 1
            OCH = 8
            ost = None
            ost_base = 0
            ost_n = 0
            b0 = 0
            while b0 < NBLK:
                gn = min(4, NBLK - b0)
                gi = b0 // 4
                pa = pa_pool.tile([D, 512], f32, tag="pa")
                scatter_group(sl_, b0, gn, pa)
                aggL = agg2pool.tile([D, 512], bf16, tag="aggL")
                nc.vector.tensor_copy(out=aggL[:], in_=pa[:])

                po = po_pool.tile([128, 512], f32, tag="po")
                for q in range(gn):
                    b = b0 + q
                    nb = min(128, NB - b * 128)
                    sl = slice(q * 128, q * 128 + D)
                    nc.tensor.matmul(po[:nb, sl],
                                     lhsT=xT_own[:, b * 128:b * 128 + nb],
                                     rhs=WnW[:], start=True, stop=False)
                    for s in range(NS - 1):
                        nc.tensor.matmul(po[:nb, sl],
                                         lhsT=aggs[s][gi][:, q * 128:q * 128 + nb],
                                         rhs=WgW[:], start=False, stop=False)
                    nc.tensor.matmul(po[:nb, sl],
                                     lhsT=aggL[:, q * 128:q * 128 + nb],
                                     rhs=WgW[:], start=False, stop=False)
                    nc.tensor.matmul(po[:nb, sl], lhsT=onesR[:1, :nb],
                                     rhs=biasR[:1, :], start=False, stop=True)
                for q in range(gn):
                    b = b0 + q
                    nb = min(128, NB - b * 128)
                    if ost is None:
                        ost = ospool.tile([128, OCH * D], f32, tag="ost")
                        ost_base = b
                        ost_n = 0
                    nc.scalar.activation(ost[:nb, ost_n * D:(ost_n + 1) * D],
                                         po[:nb, q * 128:q * 128 + D], AF.Tanh)
                    ost_n += 1
                    if (ost_n == OCH) or (b == NBLK - 1):
                        rows0 = ost_base * 128
                        nfull_o = ost_n if nb == 128 else ost_n - 1
                        if nfull_o:
                            nc.sync.dma_start(
                                out=out_d[rows0:rows0 + nfull_o * 128, :].rearrange(
                                    "(t p) d -> p t d", p=128),
                                in_=ost[:, :nfull_o * D].rearrange(
                                    "p (t d) -> p t d", d=D))
                        if nb != 128:
                            nc.sync.dma_start(
                                out=out_d[rows0 + nfull_o * 128:
                                          rows0 + nfull_o * 128 + nb, :],
                                in_=ost[:nb, nfull_o * D:nfull_o * D + D])
                        ost = None
                b0 += gn

    nc.compile()
    return nc


# ---------------------------------------------------------------- host entry

def _host_prep(p: P, x, edge_index, Wn_w, Wn_b, Wg_w, Wg_b, Wa_w, Wa_b):
    x = np.asarray(x, np.float32)
    xT = np.ascontiguousarray(x.T).astype(BF16)
    tS, L, Lg, per_core = prep_edges(
        p, np.asarray(edge_index)[0], np.asarray(edge_index)[1])

    shared = {
        "xT": xT,
        "WaW": np.asarray(Wa_w, np.float32).astype(BF16),
        "WaB": np.asarray(Wa_b, np.float32).reshape(p.D, 1),
        "WnW": np.asarray(Wn_w, np.float32).astype(BF16),
        "WgW": np.asarray(Wg_w, np.float32).astype(BF16),
        "biasR": (np.asarray(Wn_b, np.float32)
                  + np.asarray(Wg_b, np.float32)).reshape(1, p.D).astype(BF16),
        "onesR": np.ones((1, p.D), BF16),
        "ident": np.eye(p.D, dtype=np.float32).astype(BF16),
        "iota4": np.tile(np.arange(128, dtype=np.float32), 4)[None, :]
                 .repeat(128, 0).astype(BF16),
        "dummy_idx": np.zeros((128, 1), np.int16),
        "iotaP": np.arange(128, dtype=np.float32)[:, None]
                 .repeat(128, 1).astype(BF16),
    }
    in_maps = []
    for k in range(p.NCORES):
        m = dict(shared)
        m["xT_own"] = np.ascontiguousarray(xT[:, k * p.NB:(k + 1) * p.NB])
        m.update(per_core[k])
        in_maps.append(m)
    return tS, L, Lg, in_maps


TRACE = False      # set True (e.g. from test.py) to capture an NTFF profile
LAST = None        # last BassKernelResults, for profiling/inspection


def kernel(**inputs) -> np.ndarray:
    global LAST
    from concourse import bass_utils
    bass_utils.upload_artifacts = lambda tmpdir: "local://" + tmpdir

    p = P()
    tS, L, Lg, in_maps = _host_prep(p, **inputs)
    nc = build(p, tS, L, Lg)
    kw = dict(trace=True, trace_cores=list(range(p.NCORES))) if TRACE else {}
    res = bass_utils.run_bass_kernel_spmd(
        nc, in_maps, core_ids=list(range(p.NCORES)), **kw)
    LAST = res
    out = np.concatenate([res.results[k]["out"] for k in range(p.NCORES)], axis=0)
    return out.astype(np.float32)


# revision 36
# speedup vs baseline: 1.4953x; 1.4953x over previous
"""AttentiveFPConv GNN message-passing kernel for 8 Trainium2 NeuronCores.

Reference computation (all fp32):
    alpha = sigmoid(x[col] @ Wa_w + Wa_b)          # per-edge attention
    neigh = x[col] * alpha                          # per-edge message
    aggr  = segment_sum(neigh, row, N)              # per-node aggregation
    out   = tanh(x @ Wn_w + Wn_b + aggr @ Wg_w + Wg_b)

Key algebraic identity: alpha depends only on the source node, so
    h = x * sigmoid(x @ Wa_w + Wa_b)                # per-NODE tensor
    aggr[n] = sum_{e: row[e]=n} h[col[e]]           # gather + segment-sum

Sharding: destination-node sharding. Core k owns nodes [5000k, 5000(k+1))
and ALL edges targeting them (balanced: rows are uniform). No collective
needed: each core computes its own aggr and output slice.

Per-core pipeline:
  Phase 1: h = x*sigmoid(x@Wa+b) for ALL nodes (replicated). h stored in
           HBM partition-major as [128, T, D] so the store DMA has large
           contiguous per-partition runs (full DMA rate); the gather
           addresses node c at row (c%128)*T + c//128.
  Phase 2: dma_gather h[col] in destination-sorted edge order (4 SWDGE
           queues, 512-idx chunks: the 1024-desc ring then pipelines two
           chunks per queue); segment-sum via one-hot matmuls accumulating
           aggr^T in PSUM per 128-node block. One-hot M is built ON-CHIP:
           DVE tensor_tensor is_equal of a bf16 iota against the per-edge
           local-row stream (bcast AP), 4 tiles per op.
           Edges are split into THREE streams by col range (h segments
           written in order), so gathers for stream s can start as soon
           as segment s is in HBM -- keeps the Pool engine (the SWDGE
           descriptor generator, the critical resource) continuously fed.
           Streams also keep every int16 gather index < 32768.
  Phase 3: out = tanh(x@Wn + aggr@Wg + ones x bias) -- bias added by a
           rank-1 matmul into the same PSUM accumulation group.
"""

import numpy as np
import ml_dtypes

BF16 = ml_dtypes.bfloat16

# ---------------------------------------------------------------- parameters

class P:
    """Problem/kernel parameters (full-size defaults; shrinkable for tests)."""
    def __init__(self, N=40000, D=128, NCORES=8, SPLITS=(9984, 24960),
                 GCHUNK=512, PH1_CHUNK=2048, NQ=4, SCRATCH=16384,
                 PIECE=4992, GBUFS=18, NEGPAD=False):
        assert D == 128
        self.N, self.D, self.NCORES = N, D, NCORES
        self.NB = N // NCORES                 # nodes per core
        self.GCHUNK = GCHUNK                  # idxs per dma_gather call
        self.GT = GCHUNK // 128               # gather tiles per chunk
        self.PH1_CHUNK = PH1_CHUNK            # nodes per phase-1 xT chunk
        self.NBLK = (self.NB + 127) // 128    # 128-node blocks per core
        self.NQ = NQ                          # SWDGE queues for dma_gather
        self.SCRATCH = SCRATCH                # SWDGE desc ring bytes/partition
        self.PIECE = PIECE                    # nodes per phase-1 staging piece
        self.GBUFS = GBUFS                    # gather tile-pool depth per stream
        self.NEGPAD = NEGPAD                  # use idx=-1 (skipped) for padding
        # h segments: [lo, hi) node ranges, each with partition-major tiles
        bounds = (0,) + tuple(SPLITS) + (N,)
        self.SEG = []
        for lo, hi in zip(bounds[:-1], bounds[1:]):
            T = (hi - lo + 127) // 128
            assert 127 * T + (T - 1) < 32768, "int16 gather index overflow"
            self.SEG.append((lo, hi, T))
        self.NSEG = len(self.SEG)


# ------------------------------------------------------------ host edge prep

def prep_edges(p: P, row: np.ndarray, col: np.ndarray):
    """Per-core destination-sorted, block-padded edge streams (one per h
    segment). Returns per-stream tile counts and per-core idx/lr arrays."""
    row = np.asarray(row).astype(np.int64)
    col = np.asarray(col).astype(np.int64)
    NS = p.NSEG
    cores = []
    for k in range(p.NCORES):
        sel = (row // p.NB) == k
        r = (row[sel] - k * p.NB).astype(np.int32)
        c = col[sel].astype(np.int32)
        order = np.argsort(r, kind="stable")
        r, c = r[order], c[order]
        lo = np.searchsorted(r, np.arange(p.NBLK) * 128)
        hi = np.searchsorted(r, np.minimum(np.arange(1, p.NBLK + 1) * 128, p.NB))
        blocks = []
        for b in range(p.NBLK):
            rb = r[lo[b]:hi[b]] - b * 128
            cb = c[lo[b]:hi[b]]
            per_stream = []
            for (slo, shi, T) in p.SEG:
                m = (cb >= slo) & (cb < shi)
                cs = cb[m] - slo
                # partition-major h layout: node c -> row (c%128)*T + c//128
                per_stream.append(((cs % 128) * T + cs // 128, rb[m]))
            blocks.append(per_stream)
        cores.append(blocks)

    cnt = np.array([[[len(cores[k][b][s][0]) for b in range(p.NBLK)]
                     for s in range(NS)] for k in range(p.NCORES)])
    tS = np.maximum(1, -(-cnt.max(axis=0) // 128))          # [NS, NBLK]
    L = [int(tS[s].sum()) * 128 for s in range(NS)]
    Lg = [-(-L[s] // p.GCHUNK) * p.GCHUNK for s in range(NS)]

    per_core = []
    for k in range(p.NCORES):
        m = {}
        for s in range(NS):
            # Padding slots use idx=-1 (descriptor skipped by the gather HW;
            # the matching one-hot row is all-zero so stale SBUF data is
            # harmless). The first GBUFS chunks instead pad with idx=0 so
            # every gather-pool buffer gets fully written once -- later
            # skipped slots then hold stale-but-finite h values, never
            # uninitialized SBUF (which could be NaN and poison 0*NaN).
            idx = np.full(Lg[s], -1, np.int16)
            first = min(Lg[s], p.GBUFS * p.GCHUNK) if p.NEGPAD else Lg[s]
            idx[:first] = 0
            lr = np.full(L[s], -1.0, np.float32)
            o = 0
            for b in range(p.NBLK):
                cs, rs = cores[k][b][s]
                idx[o:o + len(cs)] = cs
                lr[o:o + len(rs)] = rs
                o += int(tS[s][b]) * 128
            # A chunk with zero descriptors may never fire its completion
            # semaphore -- make sure every chunk gathers at least one row.
            ch = idx.reshape(-1, p.GCHUNK)
            empty = (ch >= 0).sum(axis=1) == 0
            ch[empty] = 0
            m[f"idx{s}"] = np.tile(idx.reshape(-1, 16).T, (8, 1))
            m[f"lr{s}"] = lr.reshape(-1, 128).T.astype(BF16)
        per_core.append(m)
    return tS, L, Lg, per_core


# ------------------------------------------------------------- device kernel

def build(p: P, tS, L, Lg):
    from concourse import bacc, mybir, tile

    f32, bf16, i16 = mybir.dt.float32, mybir.dt.bfloat16, mybir.dt.int16
    AF = mybir.ActivationFunctionType
    nc = bacc.Bacc("TRN2", target_bir_lowering=False, debug=False,
                   num_devices=p.NCORES, num_swdge_queues=p.NQ,
                   dynamic_dma_scratch_size=p.SCRATCH)

    N, D, NB, NBLK, NS = p.N, p.D, p.NB, p.NBLK, p.NSEG
    PIECE = p.PIECE
    S = [L[s] // 128 for s in range(NS)]      # one-hot tiles per stream

    xT_d   = nc.dram_tensor("xT", [D, N], bf16, kind="ExternalInput")
    xTo_d  = nc.dram_tensor("xT_own", [D, NB], bf16, kind="ExternalInput")
    WaW_d  = nc.dram_tensor("WaW", [D, D], bf16, kind="ExternalInput")
    WaB_d  = nc.dram_tensor("WaB", [D, 1], f32, kind="ExternalInput")
    WnW_d  = nc.dram_tensor("WnW", [D, D], bf16, kind="ExternalInput")
    WgW_d  = nc.dram_tensor("WgW", [D, D], bf16, kind="ExternalInput")
    bias_d = nc.dram_tensor("biasR", [1, D], bf16, kind="ExternalInput")
    ones_d = nc.dram_tensor("onesR", [1, D], bf16, kind="ExternalInput")
    ident_d= nc.dram_tensor("ident", [D, D], bf16, kind="ExternalInput")
    iota_d = nc.dram_tensor("iota4", [128, 512], bf16, kind="ExternalInput")
    idx_d  = [nc.dram_tensor(f"idx{s}", [128, Lg[s] // 16], i16,
                             kind="ExternalInput") for s in range(NS)]
    lr_d   = [nc.dram_tensor(f"lr{s}", [128, S[s]], bf16,
                             kind="ExternalInput") for s in range(NS)]
    out_d  = nc.dram_tensor("out", [NB, D], f32, kind="ExternalOutput")
    h_d    = [nc.dram_tensor(f"h{s}", [128, p.SEG[s][2], D], bf16,
                             kind="Internal") for s in range(NS)]
    h_rows = [h_d[s][:, :, :].rearrange("p t d -> (p t) d") for s in range(NS)]

    from contextlib import ExitStack
    with tile.TileContext(nc) as tc:
        with ExitStack() as stack:
            pool = lambda name, bufs, **kw: stack.enter_context(
                tc.tile_pool(name=name, bufs=bufs, **kw))
            cpool = pool("const", 1)
            xpool = pool("xchunk", 3)
            htpool = pool("hT", 2)
            hspool = pool("hstage", 2)
            pg_pool = pool("pg", 2, space="PSUM")
            pt_pool = pool("pt", 1, space="PSUM")
            pa_pool = pool("pa", 2, space="PSUM")
            po_pool = pool("po", 2, space="PSUM")
            g0pool = pool("s0", p.GBUFS)
            g1pool = pool("s1", p.GBUFS)
            g2pool = pool("s2", p.GBUFS)
            mpool = pool("m", 12)
            agg0pool = pool("agg0", (NBLK + 3) // 4)
            agg1pool = pool("agg1", (NBLK + 3) // 4)
            agg2pool = pool("agg2", 3)
            w1pool = pool("ph1w", 4)
            ospool = pool("ostage", 2)
            gpools = [g0pool, g1pool, g2pool]
            aggpools = [agg0pool, agg1pool, agg2pool]

            # ---- constants needed by phase 1 (issued first on the SP queue)
            WaW = cpool.tile([D, D], bf16); nc.sync.dma_start(out=WaW[:], in_=WaW_d[:])
            WaB = cpool.tile([D, 1], f32); nc.sync.dma_start(out=WaB[:], in_=WaB_d[:])
            ident = cpool.tile([D, D], bf16); nc.sync.dma_start(out=ident[:], in_=ident_d[:])

            # phase-2/3 constants: issued on the Act HWDGE queue so they do
            # not delay the phase-1 x-chunk stream on the SP queue.
            WnW = cpool.tile([D, D], bf16); nc.scalar.dma_start(out=WnW[:], in_=WnW_d[:])
            WgW = cpool.tile([D, D], bf16); nc.scalar.dma_start(out=WgW[:], in_=WgW_d[:])
            biasR = cpool.tile([1, D], bf16); nc.scalar.dma_start(out=biasR[:], in_=bias_d[:])
            onesR = cpool.tile([1, D], bf16); nc.scalar.dma_start(out=onesR[:], in_=ones_d[:])
            iota4 = cpool.tile([128, 512], bf16); nc.scalar.dma_start(out=iota4[:], in_=iota_d[:])
            xT_own = cpool.tile([D, NB], bf16); nc.scalar.dma_start(out=xT_own[:], in_=xTo_d[:])
            lr_sb = []
            idx_sb = []
            for s in range(NS):
                lt = cpool.tile([128, S[s]], bf16, tag=f"lr{s}_sb")
                nc.scalar.dma_start(out=lt[:], in_=lr_d[s][:])
                lr_sb.append(lt)
                it = cpool.tile([128, Lg[s] // 16], i16, tag=f"idx{s}_sb")
                nc.scalar.dma_start(out=it[:], in_=idx_d[s][:])
                idx_sb.append(it)

            # ---- phase 1: h = x * sigmoid(x@Wa + b); hT pieces -> h[128,T,D]
            def ph1_compute(hTp, base, cn):
                """Compute hT for nodes [base, base+cn) into hTp[:, :cn]."""
                off = 0
                while off < cn:
                    w = min(p.PH1_CHUNK, cn - off)
                    xc = xpool.tile([D, p.PH1_CHUNK], bf16, tag="xc")
                    nc.sync.dma_start(out=xc[:, :w], in_=xT_d[:, base + off:base + off + w])
                    g0 = 0
                    while g0 < w:
                        gw = min(512, w - g0)
                        pg = pg_pool.tile([D, 512], f32, tag="pg")
                        nc.tensor.matmul(pg[:, :gw], lhsT=WaW[:],
                                         rhs=xc[:, g0:g0 + gw], start=True, stop=True)
                        sT = w1pool.tile([D, 512], bf16, tag="sT")
                        nc.scalar.activation(sT[:, :gw], pg[:, :gw], AF.Sigmoid,
                                             bias=WaB[:, 0:1])
                        nc.vector.tensor_tensor(out=hTp[:, off + g0:off + g0 + gw],
                                                in0=xc[:, g0:g0 + gw],
                                                in1=sT[:, :gw], op=mybir.AluOpType.mult)
                        g0 += gw
                    off += w

            def ph1_flush(hTp, h_t, t_base, cn):
                """PE-transpose hTp[:, :cn] into node-major tiles and DMA to
                h_t[:, t_base:t_base+ceil(cn/128), :] (partition-major)."""
                nfull = cn // 128
                rem = cn - nfull * 128
                if nfull:
                    hst = hspool.tile([128, PIECE // 128, 128], bf16, tag="hst")
                    t0 = 0
                    while t0 < nfull:
                        tn = min(4, nfull - t0)
                        pt = pt_pool.tile([128, 512], bf16, tag="pt")
                        for q in range(tn):
                            nc.tensor.transpose(
                                pt[:, q * 128:(q + 1) * 128],
                                hTp[:, (t0 + q) * 128:(t0 + q + 1) * 128], ident[:])
                        nc.vector.tensor_copy(
                            out=hst[:, t0:t0 + tn, :].rearrange("p t d -> p (t d)"),
                            in_=pt[:, :tn * 128])
                        t0 += tn
                    nc.sync.dma_start(out=h_t[:, t_base:t_base + nfull, :],
                                      in_=hst[:, :nfull, :])
                if rem:
                    pt = pt_pool.tile([128, 512], bf16, tag="pt")
                    nc.tensor.transpose(pt[:rem, :128], hTp[:, nfull * 128:nfull * 128 + rem],
                                        ident[:])
                    tl = w1pool.tile([128, 128], bf16, tag="tail")
                    nc.vector.tensor_copy(out=tl[:rem, :], in_=pt[:rem, :128])
                    nc.sync.dma_start(
                        out=h_t[:rem, t_base + nfull:t_base + nfull + 1, :],
                        in_=tl[:rem, :].rearrange("p (t d) -> p t d", t=1))

            for s in range(NS):
                slo, shi, T = p.SEG[s]
                base = slo
                while base < shi:
                    cn = min(PIECE, shi - base)
                    hTp = htpool.tile([D, PIECE], bf16, tag="hT")
                    ph1_compute(hTp, base, cn)
                    ph1_flush(hTp, h_d[s], (base - slo) // 128, cn)
                    base += cn

            # ---- phase 2: NS scatter passes (stream s gathers from h_d[s])
            nq_counter = [0]
            g_tiles = [[None] * (Lg[s] // p.GCHUNK) for s in range(NS)]
            MGT = 4
            m_tiles = [[None] * (-(-S[s] // MGT)) for s in range(NS)]

            def ensure_chunk(s, ci):
                if g_tiles[s][ci] is not None:
                    return
                g = gpools[s].tile([128, p.GT, D], bf16, tag=f"g{s}")
                c0 = ci * (p.GCHUNK // 16)
                nc.gpsimd.dma_gather(
                    out_ap=g[:], in_ap=h_rows[s],
                    idxs_ap=idx_sb[s][:, c0:c0 + p.GCHUNK // 16],
                    num_idxs=p.GCHUNK, num_idxs_reg=p.GCHUNK, elem_size=D,
                    queue_num=nq_counter[0] % p.NQ)
                nq_counter[0] += 1
                g_tiles[s][ci] = g

            def ensure_mchunk(s, ci):
                """Build 4 one-hot tiles on-chip: M[e,d] = (lr[e] == d)."""
                if m_tiles[s][ci] is not None:
                    return
                t0 = ci * MGT
                tn = min(MGT, S[s] - t0)
                mt = mpool.tile([128, MGT, D], bf16, tag=f"m{s}")
                nc.vector.tensor_tensor(
                    out=mt[:, :tn, :],
                    in0=iota4[:, :tn * 128].rearrange("p (t d) -> p t d", d=128),
                    in1=lr_sb[s][:, t0:t0 + tn].broadcast_to([128, tn, 128]),
                    op=mybir.AluOpType.is_equal)
                m_tiles[s][ci] = mt

            pos = [np.concatenate([[0], np.cumsum(tS[s])]).astype(int)
                   for s in range(NS)]

            def scatter_group(s, b0, gn, pa):
                """Accumulate stream s's one-hot matmuls for blocks
                [b0, b0+gn) into PSUM pa."""
                for q in range(gn):
                    b = b0 + q
                    tcnt = int(tS[s][b])
                    for j in range(tcnt):
                        g = pos[s][b] + j
                        ensure_chunk(s, g // p.GT)
                        ensure_mchunk(s, g // MGT)
                        neigh = g_tiles[s][g // p.GT][:, g % p.GT, :]
                        M = m_tiles[s][g // MGT][:, g % MGT, :]
                        nc.tensor.matmul(pa[:, q * 128:(q + 1) * 128],
                                         lhsT=neigh, rhs=M,
                                         start=(j == 0), stop=(j == tcnt - 1))

            # passes 0..NS-2 buffered in SBUF
            aggs = []
            for s in range(NS - 1):
                cur = []
                b0 = 0
                while b0 < NBLK:
                    gn = min(4, NBLK - b0)
                    pa = pa_pool.tile([D, 512], f32, tag="pa")
                    scatter_group(s, b0, gn, pa)
                    agg = aggpools[s].tile([D, 512], bf16, tag=f"agg{s}")
                    nc.vector.tensor_copy(out=agg[:], in_=pa[:])
                    cur.append(agg)
                    b0 += gn
                aggs.append(cur)

            # ---- last pass + phase 3 fused per 4-block group
            sl_ = NS - 1
            OCH = 8
            ost = None
            ost_base = 0
            ost_n = 0
            b0 = 0
            while b0 < NBLK:
                gn = min(4, NBLK - b0)
                gi = b0 // 4
                pa = pa_pool.tile([D, 512], f32, tag="pa")
                scatter_group(sl_, b0, gn, pa)
                aggL = agg2pool.tile([D, 512], bf16, tag="aggL")
                nc.vector.tensor_copy(out=aggL[:], in_=pa[:])

                po = po_pool.tile([128, 512], f32, tag="po")
                for q in range(gn):
                    b = b0 + q
                    nb = min(128, NB - b * 128)
                    sl = slice(q * 128, q * 128 + D)
                    nc.tensor.matmul(po[:nb, sl],
                                     lhsT=xT_own[:, b * 128:b * 128 + nb],
                                     rhs=WnW[:], start=True, stop=False)
                    for s in range(NS - 1):
                        nc.tensor.matmul(po[:nb, sl],
                                         lhsT=aggs[s][gi][:, q * 128:q * 128 + nb],
                                         rhs=WgW[:], start=False, stop=False)
                    nc.tensor.matmul(po[:nb, sl],
                                     lhsT=aggL[:, q * 128:q * 128 + nb],
                                     rhs=WgW[:], start=False, stop=False)
                    nc.tensor.matmul(po[:nb, sl], lhsT=onesR[:1, :nb],
                                     rhs=biasR[:1, :], start=False, stop=True)
                for q in range(gn):
                    b = b0 + q
                    nb = min(128, NB - b * 128)
                    if ost is None:
                        ost = ospool.tile([128, OCH * D], f32, tag="ost")
                        ost_base = b
                        ost_n = 0
                    nc.scalar.activation(ost[:nb, ost_n * D:(ost_n + 1) * D],
                                         po[:nb, q * 128:q * 128 + D], AF.Tanh)
                    ost_n += 1
                    if (ost_n == OCH) or (b == NBLK - 1):
                        rows0 = ost_base * 128
                        nfull_o = ost_n if nb == 128 else ost_n - 1
                        if nfull_o:
                            nc.sync.dma_start(
                                out=out_d[rows0:rows0 + nfull_o * 128, :].rearrange(
                                    "(t p) d -> p t d", p=128),
                                in_=ost[:, :nfull_o * D].rearrange(
                                    "p (t d) -> p t d", d=D))
                        if nb != 128:
                            nc.sync.dma_start(
                                out=out_d[rows0 + nfull_o * 128:
                                          rows0 + nfull_o * 128 + nb, :],
                                in_=ost[:nb, nfull_o * D:nfull_o * D + D])
                        ost = None
                b0 += gn

    nc.compile()
    return nc


# ---------------------------------------------------------------- host entry

def _host_prep(p: P, x, edge_index, Wn_w, Wn_b, Wg_w, Wg_b, Wa_w, Wa_b):
    x = np.asarray(x, np.float32)
    xT = np.ascontiguousarray(x.T).astype(BF16)
    tS, L, Lg, per_core = prep_edges(
        p, np.asarray(edge_index)[0], np.asarray(edge_index)[1])

    shared = {
        "xT": xT,
        "WaW": np.asarray(Wa_w, np.float32).astype(BF16),
        "WaB": np.asarray(Wa_b, np.float32).reshape(p.D, 1),
        "WnW": np.asarray(Wn_w, np.float32).astype(BF16),
        "WgW": np.asarray(Wg_w, np.float32).astype(BF16),
        "biasR": (np.asarray(Wn_b, np.float32)
                  + np.asarray(Wg_b, np.float32)).reshape(1, p.D).astype(BF16),
        "onesR": np.ones((1, p.D), BF16),
        "ident": np.eye(p.D, dtype=np.float32).astype(BF16),
        "iota4": np.tile(np.arange(128, dtype=np.float32), 4)[None, :]
                 .repeat(128, 0).astype(BF16),
    }
    in_maps = []
    for k in range(p.NCORES):
        m = dict(shared)
        m["xT_own"] = np.ascontiguousarray(xT[:, k * p.NB:(k + 1) * p.NB])
        m.update(per_core[k])
        in_maps.append(m)
    return tS, L, Lg, in_maps


TRACE = False      # set True (e.g. from test.py) to capture an NTFF profile
LAST = None        # last BassKernelResults, for profiling/inspection


def kernel(**inputs) -> np.ndarray:
    global LAST
    from concourse import bass_utils
    bass_utils.upload_artifacts = lambda tmpdir: "local://" + tmpdir

    p = P()
    tS, L, Lg, in_maps = _host_prep(p, **inputs)
    nc = build(p, tS, L, Lg)
    kw = dict(trace=True, trace_cores=list(range(p.NCORES))) if TRACE else {}
    res = bass_utils.run_bass_kernel_spmd(
        nc, in_maps, core_ids=list(range(p.NCORES)), **kw)
    LAST = res
    out = np.concatenate([res.results[k]["out"] for k in range(p.NCORES)], axis=0)
    return out.astype(np.float32)
